# revision 3
# baseline (speedup 1.0000x reference)
"""AdaptiveCompressionLayer kernel for 8 TRN2 NeuronCores.

Strategy (expert-routed data parallel, collapsed experts):
  - Host: bucket tokens by importance score (>0.8 / >0.4 / rest), gather
    tokens into per-expert groups, split evenly across 8 cores with fixed
    per-expert capacities, and pre-transpose the routed activations to
    [H, T_pad] so the device needs no on-chip transposes.
  - Experts 0/1 have hc (691/537) > H/2, so the two-stage
    compress->decompress is MORE flops than the collapsed single matmul:
    host-precompute W_e = Wc_e @ Wd_e  [H, H]  and  b_e = bc_e@Wd_e + bd_e.
    Device then does one dense [128tok x 768] = x^T.T @ W per subtile
    (6 K-chunks, N split 512+256 for PSUM banks) — full PE utilization,
    no ragged tails.  The bias b_e is folded in with a K=1 ones-row
    matmul (rhs = b_e broadcast row); two subtiles' bias rows run
    concurrently as row-tiles at partition offsets 0/32.
  - Expert 2 (hc=76 < H/2) keeps the factored two-stage path:
        Z^T = Wc2^T @ X^T  (one M=76 chunk), += bc2 via ACT copy,
        Y   = Z^T.T @ [Wd2; bd2]  (ones-row trick folds bd2)
  - LayerNorm per 128-token subtile: bn_stats/bn_aggr (DVE), sqrt (ACT),
    reciprocal (DVE), normalize copy psum->sbuf bf16 on ACT with
    per-partition scale/bias.  Output stores go out on the GpSimd DMA
    queue so they never stall ACT/DVE.
  - Host: scatter valid rows back to the original token order.

No cross-core communication: routing is per-token, weights replicated.
"""
import sys

sys.path.insert(0, "/opt/trn_rl_repo")

import numpy as np
import ml_dtypes

BF16 = ml_dtypes.bfloat16

H = 768
HC2 = 76  # expert-2 bottleneck (only expert kept factored)
S = 65536
EPS = 1e-5
N_CORES = 8
GROUP = 512
CAPS = (1792, 3328, 3328)  # default; kernel() tightens from actual counts

TRACE = False
LAST_RESULT = None

_NC_CACHE = {}


def _groups(caps):
    per_e = []
    offs = (0, caps[0], caps[0] + caps[1])
    for e in range(3):
        glist = []
        t = 0
        while t < caps[e]:
            gsz = min(GROUP, caps[e] - t)
            glist.append((e, offs[e] + t, gsz))
            t += gsz
        per_e.append(glist)
    return per_e


def _order(caps):
    """Flat group order: e2 first (tiny weights arrive first so PE can
    start early), then PE-heavy e0/e1 groups with DVE-heavy e2 groups
    spread between them (an e2 group has more LayerNorm work than its
    own matmuls can hide); ends on the smallest group so the final
    LayerNorm+store tail is short."""
    per_e = _groups(caps)
    e0g, e1g, e2g = per_e[0], per_e[1], per_e[2]
    heavy = []
    h1, h0 = list(e1g), list(e0g)
    # keep one smallest (any sub-512) heavy group for the very end
    tail_small = [g for g in h0 + h1 if g[2] < 512][-1:]
    h0 = [g for g in h0 if g not in tail_small]
    h1 = [g for g in h1 if g not in tail_small]
    ratio = max(1, len(h1) // max(1, len(h0)))
    while h1 or h0:
        for _ in range(ratio):
            if h1:
                heavy.append(h1.pop(0))
        if h0:
            heavy.append(h0.pop(0))
    heavy.extend(tail_small)
    light = list(e2g)
    order = []
    if light:
        order.append(light.pop(0))
    hi = 0
    # two heavies up front (weights for e1 arrive during e2+e1 work),
    # then alternate heavy/light until lights run out
    nburst = 2
    while hi < len(heavy) or light:
        for _ in range(nburst):
            if hi < len(heavy):
                order.append(heavy[hi])
                hi += 1
        nburst = 1
        if light:
            order.append(light.pop(0))
    return order


def _first_use_order(caps):
    seen = []
    for e, _, _ in _order(caps):
        if e not in seen:
            seen.append(e)
    return seen


def _build(apply_gb: bool, caps=CAPS):
    import concourse.mybir as mybir
    import concourse.tile as tile
    from concourse import bacc

    f32 = mybir.dt.float32
    bf16 = mybir.dt.bfloat16
    AF = mybir.ActivationFunctionType
    ALU = mybir.AluOpType

    tpad = sum(caps)

    nc = bacc.Bacc(None, target_bir_lowering=False)

    xt_d = nc.declare_dram_parameter("xt", [H, tpad], bf16, isOutput=False)
    # collapsed expert weights [128, 6*H]: [p, c*H+h] = W_e[c*128+p, h]
    w_d = [
        nc.declare_dram_parameter(f"w{e}", [128, 6 * H], bf16, isOutput=False)
        for e in range(2)
    ]
    wc2_d = nc.declare_dram_parameter("wc2", [128, 6 * HC2], bf16, isOutput=False)
    wd2_d = nc.declare_dram_parameter("wd2", [128, H], bf16, isOutput=False)
    bc2_d = nc.declare_dram_parameter("bc2", [128, 1], f32, isOutput=False)
    bb_d = nc.declare_dram_parameter("bb", [2, H], bf16, isOutput=False)
    if apply_gb:
        gb_d = nc.declare_dram_parameter("gb", [2, H], f32, isOutput=False)
    out_d = nc.declare_dram_parameter("out", [tpad, H], bf16, isOutput=True)

    with tile.TileContext(nc) as tc:
        from contextlib import ExitStack

        with ExitStack() as ctx:
            wpool = ctx.enter_context(tc.tile_pool(name="weights", bufs=1))
            cpool = ctx.enter_context(tc.tile_pool(name="consts", bufs=1))
            xpool = ctx.enter_context(tc.tile_pool(name="xt", bufs=8))
            zpsum = ctx.enter_context(tc.tile_pool(name="zpsum", bufs=2, space="PSUM"))
            zpool = ctx.enter_context(tc.tile_pool(name="zsb", bufs=4))
            ypsum = ctx.enter_context(tc.tile_pool(name="ypsum", bufs=3, space="PSUM"))
            opool = ctx.enter_context(tc.tile_pool(name="osb", bufs=4))
            lnpool = ctx.enter_context(tc.tile_pool(name="ln", bufs=8))

            # ---- constants first (tiny; first z-copy needs bc2) ----
            bc2_sb = cpool.tile([128, 1], f32)
            nc.scalar.dma_start(out=bc2_sb, in_=bc2_d[:, :])
            eps_t = cpool.tile([128, 1], f32)
            nc.vector.memset(eps_t, EPS)
            ones_t = cpool.tile([128, 128], bf16)
            nc.gpsimd.memset(ones_t, 1.0)
            bb_sb = cpool.tile([128, 2, H], bf16)
            nc.scalar.dma_start(out=bb_sb, in_=bb_d.ap().partition_broadcast(128))
            if apply_gb:
                gb_sb = cpool.tile([128, 2, H], f32)
                nc.scalar.dma_start(
                    out=gb_sb,
                    in_=gb_d.ap().partition_broadcast(128),
                )

            # ---- weight tiles; host-prepacked [128, ...] images ----
            w_sb = [None] * 2
            for e in range(2):
                w_sb[e] = wpool.tile([128, 6, H], bf16, tag=f"w{e}", name=f"w_sb{e}")
            wc2_sb = wpool.tile([128, 6, HC2], bf16, tag="wc2", name="wc2_sb")
            wd2_sb = wpool.tile([128, H], bf16, tag="wd2", name="wd2_sb")

            def _issue_w(e, eng):
                eng.dma_start(
                    out=w_sb[e],
                    in_=w_d[e].ap().rearrange("p (c h) -> p c h", c=6),
                )

            def _issue_wc2(eng):
                eng.dma_start(
                    out=wc2_sb,
                    in_=wc2_d.ap().rearrange("p (c h) -> p c h", c=6),
                )

            def _issue_wd2(eng):
                eng.dma_start(out=wd2_sb, in_=wd2_d[:, :])

            # weights go on the gpsimd queue (separate from xt's sync
            # queue) so big W loads overlap token loads.  e2's tiny
            # weights first so the first group can start immediately.
            order_e = _first_use_order(caps)
            pending_weights = []
            for e in order_e:
                if e == 2:
                    pending_weights.append(lambda: _issue_wc2(nc.gpsimd))
                    pending_weights.append(lambda: _issue_wd2(nc.gpsimd))
                else:
                    pending_weights.append(lambda e2=e: _issue_w(e2, nc.gpsimd))
            pending_weights.pop(0)()

            # PE warm-up: dummy matmuls during the initial weight DMA wait
            # keep the HAM activity window hot so real matmuls start at
            # full clock.
            warm = cpool.tile([128, 512], bf16, name="warm")
            nc.vector.memset(warm, 0.0)
            warm_ps = zpsum.tile([128, 512], f32, tag="pz", name="warm_ps")
            for _w in range(16):
                nc.tensor.matmul(
                    warm_ps,
                    lhsT=warm[:, 0:128],
                    rhs=warm,
                    start=(_w == 0),
                    stop=(_w == 15),
                )
            xt_r = xt_d.ap().rearrange("(c p) t -> p c t", p=128)

            subtile_no = 0

            def do_ln(py, o_t):
                nonlocal subtile_no
                stats = lnpool.tile([128, 2, 6], f32, tag="stats")
                for j in range(2):
                    nc.vector.bn_stats(
                        out=stats[:, j, :], in_=py[:, j * 384 : (j + 1) * 384]
                    )
                mv = lnpool.tile([128, 2], f32, tag="mv")
                nc.vector.bn_aggr(out=mv, in_=stats)
                rstd = lnpool.tile([128, 1], f32, tag="rstd")
                nc.scalar.activation(
                    out=rstd, in_=mv[:, 1:2], func=AF.Sqrt, bias=eps_t, scale=1.0
                )
                nc.vector.reciprocal(out=rstd, in_=rstd)
                subtile_no += 1
                if subtile_no % 5 != 0:
                    negmu = lnpool.tile([128, 1], f32, tag="negmu")
                    nc.vector.tensor_scalar(
                        out=negmu,
                        in0=mv[:, 0:1],
                        scalar1=rstd[:, 0:1],
                        scalar2=-1.0,
                        op0=ALU.mult,
                        op1=ALU.mult,
                    )
                    nc.scalar.activation(
                        out=o_t,
                        in_=py,
                        func=AF.Identity,
                        bias=negmu,
                        scale=rstd[:, 0:1],
                    )
                else:
                    nc.vector.tensor_scalar(
                        out=o_t,
                        in0=py,
                        scalar1=mv[:, 0:1],
                        scalar2=rstd[:, 0:1],
                        op0=ALU.subtract,
                        op1=ALU.mult,
                    )
                if apply_gb:
                    nc.gpsimd.tensor_tensor(
                        out=o_t, in0=o_t, in1=gb_sb[:, 0, :], op=ALU.mult
                    )
                    nc.vector.tensor_add(o_t, o_t, gb_sb[:, 1, :])

            order = _order(caps)
            for grp_no, (e, tok0, gsz) in enumerate(order):
                xt_t = xpool.tile([128, 6, gsz], bf16, tag="xt")
                nc.sync.dma_start(out=xt_t, in_=xt_r[:, :, tok0 : tok0 + gsz])
                if pending_weights:
                    pending_weights.pop(0)()
                nsub = gsz // 128
                o_g = opool.tile([128, nsub, H], bf16, tag="o")
                if e == 2:
                    # ---- factored path: mm1 (M=76) then mm2 (K=77) ----
                    zt = zpool.tile([128, 1, gsz], bf16, tag="zt")
                    # ones rows for the bd2 term: memset the 32-aligned
                    # window covering partition 76; the z copy below
                    # overwrites rows 0..75 inside it.
                    nc.gpsimd.memset(zt[64:96, 0, :], 1.0)
                    pz = zpsum.tile([128, gsz], f32, tag="pz")
                    for c in range(6):
                        nc.tensor.matmul(
                            pz[0:HC2, :],
                            lhsT=wc2_sb[:, c, :],
                            rhs=xt_t[:, c, :],
                            start=(c == 0),
                            stop=(c == 5),
                        )
                    nc.scalar.activation(
                        out=zt[0:HC2, 0, :],
                        in_=pz[0:HC2, :],
                        func=AF.Identity,
                        bias=bc2_sb[0:HC2, 0:1],
                        scale=1.0,
                    )
                    for s in range(nsub):
                        py = ypsum.tile([128, H], f32, tag="py")
                        for n0, nn in ((0, 512), (512, 256)):
                            nc.tensor.matmul(
                                py[:, n0 : n0 + nn],
                                lhsT=zt[0 : HC2 + 1, 0, s * 128 : (s + 1) * 128],
                                rhs=wd2_sb[0 : HC2 + 1, n0 : n0 + nn],
                                start=True,
                                stop=True,
                            )
                        do_ln(py, o_g[:, s, :])
                else:
                    # ---- collapsed path: one [128tok, H] matmul/subtile,
                    # subtiles in pairs whose K=1 bias rows run
                    # concurrently as row-tiles at offsets 0/32 ----
                    for s0 in range(0, nsub, 2):
                        pair = [s0] + ([s0 + 1] if s0 + 1 < nsub else [])
                        pys = []
                        for idx, s in enumerate(pair):
                            py = ypsum.tile([128, H], f32, tag="py")
                            pys.append((s, py, 32 * idx))
                            for c in range(6):
                                for n0, nn in ((0, 512), (512, 256)):
                                    nc.tensor.matmul(
                                        py[:, n0 : n0 + nn],
                                        lhsT=xt_t[:, c, s * 128 : (s + 1) * 128],
                                        rhs=w_sb[e][:, c, n0 : n0 + nn],
                                        start=(c == 0),
                                        stop=False,
                                    )
                        for n0, nn in ((0, 512), (512, 256)):
                            for s, py, r in pys:
                                nc.tensor.matmul(
                                    py[:, n0 : n0 + nn],
                                    lhsT=ones_t[r : r + 1, 0:128],
                                    rhs=bb_sb[r : r + 1, e, n0 : n0 + nn],
                                    start=False,
                                    stop=True,
                                    tile_position=(r, 0),
                                )
                        for s, py, _r in pys:
                            do_ln(py, o_g[:, s, :])
                nc.gpsimd.dma_start(
                    out=out_d[tok0 : tok0 + gsz, :].rearrange(
                        "(s p) h -> p s h", p=128
                    ),
                    in_=o_g,
                )
    nc.finalize()
    return nc


def _get_nc(apply_gb: bool, caps):
    key = (apply_gb, caps)
    if key not in _NC_CACHE:
        _NC_CACHE[key] = _build(apply_gb, caps=caps)
    return _NC_CACHE[key]


def _pack_weights(inputs):
    base = {}
    # collapsed experts 0/1: W = Wc@Wd, b = bc@Wd + bd (f64 precompute)
    for e in range(2):
        wc = np.asarray(inputs[f"Wc{e}"], dtype=np.float64)
        bc = np.asarray(inputs[f"bc{e}"], dtype=np.float64)
        wd = np.asarray(inputs[f"Wd{e}"], dtype=np.float64)
        bd = np.asarray(inputs[f"bd{e}"], dtype=np.float64)
        W = wc @ wd  # [H, H]
        b = bc @ wd + bd  # [H]
        wi = W.reshape(6, 128, H).transpose(1, 0, 2)  # [p, c, h]
        base[f"w{e}"] = np.ascontiguousarray(wi.reshape(128, 6 * H)).astype(BF16)
        base.setdefault("_bb", np.zeros((2, H), np.float64))
        base["_bb"][e] = b
    base["bb"] = np.ascontiguousarray(base.pop("_bb")).astype(BF16)
    # expert 2 factored
    wc2 = np.asarray(inputs["Wc2"], dtype=np.float32)  # [H, 76]
    bc2 = np.asarray(inputs["bc2"], dtype=np.float32)
    wd2 = np.asarray(inputs["Wd2"], dtype=np.float32)  # [76, H]
    bd2 = np.asarray(inputs["bd2"], dtype=np.float32)
    wc2i = wc2.reshape(6, 128, HC2).transpose(1, 0, 2)
    base["wc2"] = np.ascontiguousarray(wc2i.reshape(128, 6 * HC2)).astype(BF16)
    wd2i = np.zeros((128, H), np.float32)
    wd2i[0:HC2] = wd2
    wd2i[HC2] = bd2
    base["wd2"] = np.ascontiguousarray(wd2i).astype(BF16)
    bc2p = np.zeros((128, 1), np.float32)
    bc2p[0:HC2, 0] = bc2
    base["bc2"] = bc2p
    return base


def kernel(**inputs):
    global LAST_RESULT
    from concourse.bass_utils import run_bass_kernel_spmd

    hs = np.ascontiguousarray(np.asarray(inputs["hidden_states"], dtype=np.float32))
    sc = np.asarray(inputs["importance_scores"], dtype=np.float32)
    gamma = np.asarray(inputs["gamma"], dtype=np.float32)
    beta = np.asarray(inputs["beta"], dtype=np.float32)

    # routing (must match f32 comparison semantics of the reference)
    m0 = sc > np.float32(0.8)
    m1 = (sc > np.float32(0.4)) & ~m0
    bucket = np.where(m0, 0, np.where(m1, 1, 2)).astype(np.int64)
    idx = [np.flatnonzero(bucket == e) for e in range(3)]
    splits = [np.array_split(idx[e], N_CORES) for e in range(3)]

    # tight per-core caps: max per-core count rounded up to 128
    caps = tuple(
        int(-(-max(len(p) for p in splits[e]) // 128) * 128) for e in range(3)
    )
    tpad = sum(caps)
    offs = (0, caps[0], caps[0] + caps[1])

    gidx = np.zeros((N_CORES, tpad), np.int64)
    valid = np.zeros((N_CORES, tpad), bool)
    for c in range(N_CORES):
        for e in range(3):
            p = splits[e][c]
            o = offs[e]
            gidx[c, o : o + len(p)] = p
            valid[c, o : o + len(p)] = True

    apply_gb = not (np.all(gamma == 1.0) and np.all(beta == 0.0))
    nc = _get_nc(apply_gb, caps)

    base = _pack_weights(inputs)
    if apply_gb:
        base["gb"] = np.ascontiguousarray(np.stack([gamma, beta], axis=0))

    in_maps = []
    for c in range(N_CORES):
        xc = hs[gidx[c]]  # [TPAD, H]
        m = dict(base)
        m["xt"] = np.ascontiguousarray(xc.T.astype(BF16))
        in_maps.append(m)

    # The device occasionally returns corrupted (non-finite) results or
    # raises an unrecoverable-state error; inputs are finite and LayerNorm
    # output is always finite, so retry in both cases.
    for attempt in range(4):
        try:
            res = run_bass_kernel_spmd(
                nc, in_maps, core_ids=list(range(N_CORES)), trace=TRACE
            )
        except Exception:
            if attempt == 3:
                raise
            import time as _time

            _time.sleep(2.0)
            continue
        LAST_RESULT = res
        out = np.empty((S, H), np.float32)
        for c in range(N_CORES):
            v = valid[c]
            out[gidx[c][v]] = res.results[c]["out"][v]
        if np.isfinite(out).all():
            break
    return out
